# revision 1
# baseline (speedup 1.0000x reference)
"""Inverse DWT (BackwardTransformLayer) Trainium2 Bass kernel.

Math (polyphase form of the zero-interleaved circular FFT convolution):
  out[r, 2p+pi] = sum_{s=0..3} cD[pi,s]*D[r,(p-s)%M] + cA[pi,s]*A[r,(p-s)%M]
  cD[0,s] = w[7-2s]   cD[1,s] = w[6-2s]   cA[0,s] = w[2s]   cA[1,s] = -w[2s+1]

Sharding: data-parallel over rows; 512 rows per core on 8 NeuronCores
(run_bass_kernel_spmd). Full fp32 inputs in, full fp32 output out.

Per core, each 128-row tile is loaded once into SBUF with a 3-column
circular prefix. The 32 (row-tile, parity, 1024-col panel) work units
split 22 TensorE / 10 VectorE, panel-paired so both parities of a DVE
panel share inputs:
  - PE: float32r identity-scaled matmuls (1 cyc/row at free>=256), the 8
    taps accumulating in a PSUM bank per 512-col chunk, evicted stride-2
    by ScalarE into the interleaved output tile.
  - DVE: scalar_tensor_tensor MAC chains (first product on ScalarE, last
    MAC written stride-2 into the output tile), reading PRIVATE copies of
    its input panels loaded by extra DMAs. Sharing the PE's input tiles
    caps the mix at ~119us (SBUF contention: marginal DVE value decays to
    0 past ~7 units); private copies reach 103-114us, and DMA has the
    slack (IO-only floor is 73.6us).
  - DMA: loads on qPool (gpsimd SWDGE), stores alternating qSP/qAct
    HWDGE; queues run in parallel on real HW (all-qSP floor is 108us).

Measured (in-kernel repetition slope, per core): 103-114us vs the 202-210us
fp32 baseline; rel err 1.7e-4 (f32r mantissa) vs the 2e-2 gate. Exact-fp32
fallback: DWT_DTYPE=f32 (rel err 2.9e-7, ~1.7x slower). Negative results,
kept off the default path: bf16 compute (DVE 16-bit 2x modes do not engage
on HW; chain-accumulation err 8e-3), InstLdweights dedupe (HW already hides
weight reloads in the background weight buffer; removing them is ~15%
slower), GPSIMD compute units (SBUF port shared with DVE). The unit
placement is pipeline-sensitive: the DWT_COST_* knobs pick both the split
and the panel ordering, and neighbors of the default measured 130-148us.
"""

import os
import sys

import numpy as np

for _p in ("/opt/trn_rl_repo", "/root/.axon_site/_ro/trn_rl_repo"):
    if os.path.isdir(_p) and _p not in sys.path:
        sys.path.append(_p)

import concourse.bass as bass  # noqa: E402
import concourse.tile as tile  # noqa: E402
from concourse import bacc, mybir  # noqa: E402
from concourse.bass_utils import run_bass_kernel_spmd  # noqa: E402

F32 = mybir.dt.float32
F32R = mybir.dt.float32r
BF16 = mybir.dt.bfloat16
COPY = mybir.ActivationFunctionType.Copy
MUL = mybir.AluOpType.mult
ADD = mybir.AluOpType.add

N_CORES = 8
P = 128          # partitions
M = 4096         # input row length
ROWS = 512       # rows per core
NT = ROWS // P   # row tiles per core
WU = int(os.environ.get("DWT_WU", "1024"))  # 512 measured WORSE (139.5us vs 114)
NPAN = M // WU   # panels per parity
NCHUNK = 512     # psum chunk (one bank of fp32)
INP_BUFS = int(os.environ.get("DWT_INP_BUFS", "2"))
OUT_BUFS = int(os.environ.get("DWT_OUT_BUFS", "3"))  # 4 measured WORSE (133us vs 114)
ACC_BUFS = int(os.environ.get("DWT_ACC_BUFS", "8"))
PSUM_BUFS = int(os.environ.get("DWT_PSUM_BUFS", "8"))
REPS = int(os.environ.get("DWT_REPS", "1"))  # benchmark-only: repeat body in-kernel
IO_ONLY = bool(int(os.environ.get("DWT_IO_ONLY", "0")))
DTYPE_MODE = os.environ.get("DWT_DTYPE", "f32r")  # f32 | f32r | bf16
QSPREAD = bool(int(os.environ.get("DWT_QSPREAD", "1")))  # stores alternate qSP/qAct
# BROKEN: gpsimd tensor_copy with a PSUM source fails at runtime
# (JaxRuntimeError INTERNAL in codegen/ucode). Keep 0.
GPS_EVICT = bool(int(os.environ.get("DWT_GPS_EVICT", "0")))  # some PSUM evicts on gpsimd
DVE_FIRST = bool(int(os.environ.get("DWT_DVE_FIRST", "0")))  # first tap on DVE not ScalarE
STORE_NCH = int(os.environ.get("DWT_STORE_NCH", str(M)))  # store chunk cols (2048 = store half-tiles in halves)
SPLIT_CHAINS = bool(int(os.environ.get("DWT_SPLIT_CHAINS", "0")))  # D/A accs separate (halves bf16 rounding depth)

CDT = {"f32": F32, "f32r": F32R, "bf16": BF16}[DTYPE_MODE]
ADT = BF16 if DTYPE_MODE == "bf16" else F32  # DVE accumulator dtype
PRIV_DVE = bool(int(os.environ.get("DWT_PRIV_DVE", "1")))  # DVE units read private input copies
PRIV_BUFS = int(os.environ.get("DWT_PRIV_BUFS", "4"))
DEDUP_LW = bool(int(os.environ.get("DWT_DEDUP_LW", "0")))  # drop redundant PE weight reloads
# measured: removing per-matmul InstLdweights SLOWS the PE ~15% — the HW hides
# weight reloads behind matmuls (background weight buffer), and bare
# back-to-back InstMatmult(ldweights=False) pipelines worse. Keep off.


def _dedupe_ldweights(nc):
    """Remove InstLdweights that reload the weights already resident in the
    PE array (bf16/fp16 builds split each matmul into InstLdweights +
    InstMatmult(ldweights=False); the array keeps the stationary operand
    across matmuls, so back-to-back reloads of the same slice are dead
    cycles — ~128 PE cycles each).

    Only removes a reload when, since the previous load of the identical
    weights AP, the PE stream contains nothing but InstMatmult: any other
    PE instruction conservatively invalidates the tracked state. A removed
    reload must carry no semaphore updates; a single wait is rehomed onto
    the following matmul when that matmul has no wait of its own (per-engine
    execution is in-order, so the wait still guards the consumer)."""
    from concourse import mybir as _mb

    pe = _mb.EngineType.PE
    removed = 0

    def sig(inst):
        a = inst.ins[0]
        return (
            getattr(a, "memref", None),
            getattr(a, "offset", None),
            str(getattr(a, "ap", None)),
            getattr(a, "dtype", None),
        )

    for f in nc.m.functions:
        for b in f.blocks:
            cur = None
            pending = None  # (lw_inst, waits) waiting for a host matmul
            out = []
            changed = False
            for inst in b.instructions:
                if getattr(inst, "engine", None) != pe:
                    out.append(inst)
                    continue
                tn = type(inst).__name__
                if tn == "InstLdweights":
                    if pending is not None:  # consecutive LWs: restore pending
                        out.append(pending[0])
                        removed -= 1
                        pending = None
                        cur = None
                    si = inst.sync_info
                    waits = list(si.on_wait) if si is not None else []
                    ups = list(si.on_update) if si is not None else []
                    s = sig(inst)
                    if s == cur and not ups and not waits:
                        # waits stay on their LW host: a wait on a matmul
                        # stalls the PE pipeline (that's why bacc moves
                        # matmul waits onto ldweights in the first place)
                        removed += 1
                        changed = True
                        continue
                    cur = s
                    out.append(inst)
                elif tn == "InstMatmult":
                    if pending is not None:
                        lw, waits = pending
                        pending = None
                        msi = inst.sync_info
                        mwaits = list(msi.on_wait) if msi is not None else []
                        if mwaits:
                            out.append(lw)  # matmul can't host the wait
                            removed -= 1
                            cur = sig(lw)
                        elif msi is None:
                            inst.sync_info = _mb.SyncInfo(on_wait=waits, on_update=[])
                        else:
                            msi.on_wait = waits
                    out.append(inst)
                else:
                    if pending is not None:
                        out.append(pending[0])
                        removed -= 1
                        pending = None
                    cur = None
                    out.append(inst)
            if pending is not None:
                out.append(pending[0])
                removed -= 1
            if changed:
                b.instructions[:] = out
    return removed


def _unit_plan():
    """Greedy engine assignment for the NT*2*NPAN work units.

    Uses measured per-unit costs so PE/DVE units interleave evenly through
    program order (a quota split with stale costs serializes the tail)."""
    force = os.environ.get("DWT_FORCE_ENG")
    if force:
        return {(t, pi, p): force for t in range(NT) for p in range(NPAN) for pi in range(2)}
    if PRIV_DVE:
        # panel-level assignment (both parities together) so a DVE panel's
        # private input copy is shared by its two parity units
        # DVE=8.0 not the measured ~10: the ratio only sets placement order
        # here, and this ordering measured 103us vs 130-139us for neighbors
        # (the pipeline is placement-sensitive; tune DWT_COST_DVE, not logic)
        cost = {
            "PE": float(os.environ.get("DWT_COST_PE", "4.3")) * WU / 1024,
            "DVE": float(os.environ.get("DWT_COST_DVE", "8.0")) * WU / 1024,
        }
        load = {"PE": 0.0, "DVE": 0.0}
        plan = {}
        for t in range(NT):
            for p in range(NPAN):
                eng = min(cost, key=lambda e: load[e] + 2 * cost[e])
                load[eng] += 2 * cost[eng]
                plan[(t, 0, p)] = plan[(t, 1, p)] = eng
        return plan
    # (PE, DVE) us per 1024-col work unit, measured on trn2. The ratio sets
    # the split (f32r -> 25 PE / 7 DVE); marginal DVE value decays past ~7
    # units (SBUF contention with PE on the shared input tiles).
    default_cost = {
        "f32": (16.4, 8.6),
        "f32r": (4.3, 15.4),
        "bf16": (4.3, 23.2),
    }[DTYPE_MODE]
    cost = {
        "PE": float(os.environ.get("DWT_COST_PE", str(default_cost[0]))) * WU / 1024,
        "DVE": float(os.environ.get("DWT_COST_DVE", str(default_cost[1]))) * WU / 1024,
        "GPS": float(os.environ.get("DWT_COST_GPS", "1000000")),
    }
    load = {"PE": 0.0, "DVE": 0.0, "GPS": 0.0}
    plan = {}
    for t in range(NT):
        for p in range(NPAN):
            for pi in range(2):
                eng = min(cost, key=lambda e: load[e] + cost[e])
                load[eng] += cost[eng]
                plan[(t, pi, p)] = eng
    return plan


def build_nc(plan):
    nc = bacc.Bacc()
    det = nc.declare_dram_parameter("details", [ROWS, M], F32, isOutput=False)
    app = nc.declare_dram_parameter("approximation", [ROWS, M], F32, isOutput=False)
    wav = nc.declare_dram_parameter("wavelet", [8], F32, isOutput=False)
    res = nc.declare_dram_parameter("result", [ROWS, 2 * M], F32, isOutput=True)
    ident = nc.inline_tensor(np.eye(P, dtype=np.float32), "ident")

    with tile.TileContext(nc) as tc:
        with (
            tc.tile_pool(name="const", bufs=1) as constp,
            tc.tile_pool(name="ine", bufs=INP_BUFS) as inp,
            tc.tile_pool(name="oute", bufs=OUT_BUFS) as outp,
            tc.tile_pool(name="acc", bufs=ACC_BUFS) as accp,
            tc.tile_pool(name="priv", bufs=PRIV_BUFS) as privp,
            tc.tile_pool(name="psum", bufs=PSUM_BUFS, space="PSUM") as psump,
        ):
            # ---- coefficients: broadcast wavelet to all partitions
            wv = constp.tile([1, 8], F32)
            nc.sync.dma_start(wv[:], wav[None, :])
            wvb = constp.tile([P, 8], F32)
            nc.gpsimd.partition_broadcast(wvb[:], wv[:])
            wvn = constp.tile([P, 8], F32)
            nc.vector.tensor_scalar(wvn[:], wvb[:], -1.0, None, MUL)

            def coeff(x, pi, s):
                # x: 0 = details, 1 = approximation; pi: 0 = even, 1 = odd
                if x == 0:
                    idx = (7 - 2 * s) if pi == 0 else (6 - 2 * s)
                    return wvb[:, idx : idx + 1]
                if pi == 0:
                    idx = 2 * s
                    return wvb[:, idx : idx + 1]
                idx = 2 * s + 1
                return wvn[:, idx : idx + 1]

            # ---- PE weights: c * I for each (input, parity, tap)
            it = constp.tile([P, P], F32)
            nc.sync.dma_start(it[:], ident[:, :])
            w16 = constp.tile([P, 16 * P], CDT)

            def wslice(x, pi, s):
                j = (x * 2 + pi) * 4 + s
                return w16[:, j * P : (j + 1) * P]

            for x in range(2):
                for pi in range(2):
                    for s in range(4):
                        nc.vector.tensor_scalar(
                            wslice(x, pi, s), it[:], coeff(x, pi, s), None, MUL
                        )

            taps = [(x, s) for x in range(2) for s in range(4)]
            store_engines = [nc.sync, nc.scalar] if QSPREAD else [nc.sync]
            n_store = [0]

            def body(_i=None):
              for t in range(NT):
                  r0 = t * P
                  if DTYPE_MODE == "f32":
                      ld = nc.sync.dma_start
                  else:
                      ld = nc.gpsimd.dma_start  # bitcast (f32r) / downconvert (bf16)
                  dext = inp.tile([P, M + 3], CDT, tag="dext")
                  ld(dext[:, 3 : M + 3], det[r0 : r0 + P, :])
                  aext = inp.tile([P, M + 3], CDT, tag="aext")
                  ld(aext[:, 3 : M + 3], app[r0 : r0 + P, :])
                  if DTYPE_MODE == "f32r":
                      nc.vector.tensor_copy(dext[:, 0:3], dext[:, M : M + 3])
                      nc.vector.tensor_copy(aext[:, 0:3], aext[:, M : M + 3])
                  else:
                      nc.scalar.copy(dext[:, 0:3], dext[:, M : M + 3])
                      nc.scalar.copy(aext[:, 0:3], aext[:, M : M + 3])
                  ext = [dext, aext]

                  priv = {}
                  if PRIV_DVE:
                      for p2 in range(NPAN):
                          if plan[(t, 0, p2)] != "DVE":
                              continue
                          c2 = p2 * WU
                          tiles = []
                          for x, src in ((0, det), (1, app)):
                              pt = privp.tile([P, WU + 3], CDT, tag=f"pv{x}")
                              if c2 >= 3:
                                  ld(pt[:, 0 : WU + 3], src[r0 : r0 + P, c2 - 3 : c2 + WU])
                              else:
                                  ld(pt[:, 3 : WU + 3], src[r0 : r0 + P, c2 : c2 + WU])
                                  ld(pt[:, 0:3], src[r0 : r0 + P, M - 3 : M])
                              tiles.append(pt)
                          priv[p2] = tiles

                  def xv(x, a, b):
                      v = ext[x][:, a:b]
                      return v.bitcast(F32) if DTYPE_MODE == "f32r" else v

                  for h in range(2):  # two output half-tiles of M cols each
                      oh = outp.tile([P, M], F32, tag="out")
                      for p in range(h * (NPAN // 2), (h + 1) * (NPAN // 2)):
                          c0 = p * WU
                          for pi in range(2):
                              eng = plan[(t, pi, p)]
                              base = 2 * c0 - h * M + pi
                              oview = oh[:, base : min(base + 2 * WU, M) : 2]
                              pv = priv.get(p) if eng == "DVE" else None

                              def dv(x, s, pv=pv, c0=c0):
                                  if pv is not None:
                                      v = pv[x][:, 3 - s : 3 - s + WU]
                                  else:
                                      v = ext[x][:, 3 - s + c0 : 3 - s + c0 + WU]
                                  return v.bitcast(F32) if DTYPE_MODE == "f32r" else v
                              if IO_ONLY:
                                  nc.scalar.copy(oview, xv(0, 3 + c0, 3 + c0 + WU))
                                  continue
                              if eng == "PE":
                                  ccs = list(range(c0, c0 + WU, NCHUNK))
                                  pss = [psump.tile([P, NCHUNK], F32, tag="ps", name=f"ps_{t}_{pi}_{p}_{ci2}") for ci2 in range(len(ccs))]
                                  for j, (x, s) in enumerate(taps):
                                      w = wslice(x, pi, s)
                                      for ci, cc in enumerate(ccs):
                                          rhs = ext[x][:, 3 - s + cc : 3 - s + cc + NCHUNK]
                                          nc.tensor.matmul(
                                              pss[ci][:], w, rhs,
                                              start=(j == 0), stop=(j == len(taps) - 1),
                                          )
                                  for ci, cc in enumerate(ccs):
                                      evb = 2 * cc - h * M + pi
                                      ev = nc.gpsimd if (GPS_EVICT and ci % 2 == 1) else nc.scalar
                                      if ev is nc.gpsimd:
                                          nc.gpsimd.tensor_copy(
                                              oh[:, evb : min(evb + 2 * NCHUNK, M) : 2], pss[ci][:]
                                          )
                                      else:
                                          nc.scalar.copy(
                                              oh[:, evb : min(evb + 2 * NCHUNK, M) : 2], pss[ci][:]
                                          )
                              elif SPLIT_CHAINS:  # DVE, two short chains: less bf16 rounding
                                  accD = accp.tile([P, WU], ADT, tag="acc")
                                  accA = accp.tile([P, WU], ADT, tag="acc")
                                  for x, acc in ((0, accD), (1, accA)):
                                      nc.scalar.activation(
                                          acc[:], dv(x, 0),
                                          COPY, scale=coeff(x, pi, 0),
                                      )
                                      for s in range(1, 4):
                                          nc.vector.scalar_tensor_tensor(
                                              acc[:], dv(x, s),
                                              coeff(x, pi, s), acc[:], MUL, ADD,
                                          )
                                  nc.vector.tensor_tensor(oview, accD[:], accA[:], ADD)
                              else:  # DVE
                                  acc = accp.tile([P, WU], ADT, tag="acc")
                                  if DVE_FIRST:
                                      nc.vector.tensor_scalar(
                                          acc[:], dv(0, 0),
                                          coeff(0, pi, 0), None, MUL,
                                      )
                                  else:
                                      nc.scalar.activation(
                                          acc[:], dv(0, 0),
                                          COPY, scale=coeff(0, pi, 0),
                                      )
                                  for x, s in taps[1:-1]:
                                      nc.vector.scalar_tensor_tensor(
                                          acc[:], dv(x, s),
                                          coeff(x, pi, s), acc[:], MUL, ADD,
                                      )
                                  x, s = taps[-1]
                                  nc.vector.scalar_tensor_tensor(
                                      oview, dv(x, s),
                                      coeff(x, pi, s), acc[:], MUL, ADD,
                                  )
                      for sc in range(0, M, STORE_NCH):
                          st = store_engines[n_store[0] % len(store_engines)]
                          n_store[0] += 1
                          st.dma_start(
                              res[r0 : r0 + P, h * M + sc : h * M + sc + STORE_NCH],
                              oh[:, sc : sc + STORE_NCH],
                          )

            if REPS == 1:
                body()
            else:
                with tc.For_i(0, REPS, 1) as _rv:
                    body(_rv)
    nc.finalize()
    if DEDUP_LW and DTYPE_MODE == "bf16":
        n = _dedupe_ldweights(nc)
        if os.environ.get("DWT_VERBOSE"):
            print(f"dedup_ldweights removed {n}")
    return nc


_CACHE = {}


def _get_nc():
    if "nc" not in _CACHE:
        _CACHE["nc"] = build_nc(_unit_plan())
    return _CACHE["nc"]


def kernel(details, approximation, wavelet):
    details = np.ascontiguousarray(np.asarray(details, dtype=np.float32))
    approximation = np.ascontiguousarray(np.asarray(approximation, dtype=np.float32))
    wavelet = np.ascontiguousarray(np.asarray(wavelet, dtype=np.float32))
    assert details.shape == (N_CORES * ROWS, M) and approximation.shape == details.shape
    assert wavelet.shape == (8,)

    in_maps = [
        {
            "details": details[c * ROWS : (c + 1) * ROWS],
            "approximation": approximation[c * ROWS : (c + 1) * ROWS],
            "wavelet": wavelet,
        }
        for c in range(N_CORES)
    ]
    trace = bool(int(os.environ.get("DWT_TRACE", "0")))
    r = run_bass_kernel_spmd(_get_nc(), in_maps, list(range(N_CORES)), trace=trace)
    _CACHE["last_results"] = r
    return np.concatenate([r.results[c]["result"] for c in range(N_CORES)], axis=0)



# revision 53
# speedup vs baseline: 1.3777x; 1.3777x over previous
"""Inverse DWT (BackwardTransformLayer) Trainium2 Bass kernel, v2.

Math (polyphase form of the zero-interleaved circular FFT convolution):
  out[r, 2p+pi] = sum_{s=0..3} cD[pi,s]*D[r,(p-s)%M] + cA[pi,s]*A[r,(p-s)%M]
  cD[0,s] = w[7-2s]   cD[1,s] = w[6-2s]   cA[0,s] = w[2s]   cA[1,s] = -w[2s+1]

Sharding: data-parallel over rows; 512 rows per core on 8 NeuronCores
(run_bass_kernel_spmd). Full fp32 inputs in, full fp32 output out.

v2 structure (vs the v1 full-row-tile kernel): all input traffic is
per-panel [128, WU+3] tiles DMA'd straight from DRAM with the 3-column
circular prefix included in the transfer (panel 0 adds a tiny wrap DMA).
This removes v1's on-chip prefix copies and the private-DVE-copy
duplication (-5 MiB DMA/core), starts compute earlier, and gives every
work unit a private SBUF tile (no PE/DVE SBUF-port contention, which is
what made v1's sim model underpredict HW by 15%). 16 panels of WU=1024
cols; 5 on DVE (placed early; DVE is slower per panel), 11 on PE; each
panel's two parity units write one [128, 2*WU] output tile stored in
halves as soon as both finish.
  - PE panels: 8 identity-scaled f32r matmuls accumulate in a PSUM bank
    per 512-col chunk (chunk-major so ScalarE evicts chunk 0 stride-2
    into the output tile while chunk 1 computes). The f32r DRAM-side
    bitcast happens in the DMA (BIR requires f32r matmul inputs to be
    produced as f32r), so loads work on any DGE queue.
  - DVE panels: ScalarE first tap, then scalar_tensor_tensor MAC chain,
    last MAC written stride-2 into the output tile.
  - Weights: even-parity c*I slices built on ScalarE, odd on DVE; a few
    dummy identity matmuls prime the PE p-state ramp.
  - Queues: loads alternate qPool (SWDGE)/qSP, stores alternate qSP/qAct
    (DVE-panel stores qSP only: on qAct they would head-block ScalarE's
    eviction stream; on qPool they regress HW by 25us+).

Measured (test.py REPS-slope, all 8 cores): 95.8us best / ~120-123us
typical across semantically-identical rebuilds (schedule-lottery
sensitivity; CoreSim predicts 90-95us for all of them), vs 124-128us
for v1 and ~202-210us for the fp32 reference port. rel err 1.75e-4
(f32r mantissa) vs the 2e-2 gate. Negative results: bf16 loads+compute
117.8us (despite halving modeled DMA; DVE 16-bit 2x modes do not engage
on HW), WU=512 (per-op overhead), inline per-chunk PE stores, moving
weight builds/first-taps into the body, SWDGE stores.
"""

import os
import sys

import numpy as np

for _p in ("/opt/trn_rl_repo", "/root/.axon_site/_ro/trn_rl_repo"):
    if os.path.isdir(_p) and _p not in sys.path:
        sys.path.append(_p)

import concourse.bass as bass  # noqa: E402
import concourse.tile as tile  # noqa: E402
from concourse import bacc, mybir  # noqa: E402
from concourse.bass_utils import run_bass_kernel_spmd  # noqa: E402

F32 = mybir.dt.float32
F32R = mybir.dt.float32r
BF16 = mybir.dt.bfloat16
COPY = mybir.ActivationFunctionType.Copy
MUL = mybir.AluOpType.mult
ADD = mybir.AluOpType.add

N_CORES = 8
P = 128          # partitions
M = 4096         # input row length
ROWS = 512       # rows per core
NT = ROWS // P   # row tiles per core
WU = int(os.environ.get("DWT_WU", "1024"))   # panel width (input cols)
NPAN_T = M // WU                             # panels per row tile
NCHUNK = 512                                 # psum chunk (one fp32 bank)
NDVE = int(os.environ.get("DWT_NDVE", "5"))  # panels on DVE (of NT*NPAN_T)
INP_BUFS = int(os.environ.get("DWT_INP_BUFS", "8"))   # PE input ring (per input)
DVE_BUFS = int(os.environ.get("DWT_DVE_BUFS", "4"))   # DVE input ring (per input)
WARMUP = int(os.environ.get("DWT_WARMUP", "6"))  # dummy matmuls to ramp PE pstate
OUT_BUFS = int(os.environ.get("DWT_OUT_BUFS", "5"))
ACC_BUFS = int(os.environ.get("DWT_ACC_BUFS", "4"))
PSUM_BUFS = int(os.environ.get("DWT_PSUM_BUFS", "7"))  # +1 bank for PE warmup
STORE_SPLIT = int(os.environ.get("DWT_STORE_SPLIT", "1"))  # DMAs per out tile
REPS = int(os.environ.get("DWT_REPS", "1"))  # benchmark-only in-kernel loop
IO_ONLY = bool(int(os.environ.get("DWT_IO_ONLY", "0")))
DTYPE_MODE = os.environ.get("DWT_DTYPE", "f32r")  # f32r | bf16
QSPREAD = bool(int(os.environ.get("DWT_QSPREAD", "1")))  # stores qSP+qAct
CDT = {"f32r": F32R, "bf16": BF16}[DTYPE_MODE]   # matmul/weight dtype
# Input tiles carry the matmul dtype; in f32r mode the DRAM source AP is
# bitcast to f32r so the transfer is dtype-matched on any DGE queue (the BIR
# verifier requires f32r matmul inputs to be produced as f32r, so the tile
# itself must be f32r). bf16 needs the SWDGE downconvert path.
LDT = CDT
LSPREAD = bool(int(os.environ.get("DWT_LSPREAD", "1")))  # loads qPool+qSP


def _unit_plan():
    """Panel -> engine map. DVE panels are placed early in the pipeline
    (DVE is the slower engine per panel; its last unit must not define the
    tail), but panel (0,0) stays on PE so PE's first work unit is the very
    first panel loaded."""
    force = os.environ.get("DWT_FORCE_ENG")
    if force:
        return {(t, p): force for t in range(NT) for p in range(NPAN_T)}
    order = [(0, 1)] + [(t, 0) for t in range(1, NT)]
    order += [(t, p) for p in range(1, NPAN_T) for t in range(NT) if (t, p) != (0, 1)]
    order += [(0, 0)]
    dve = set(order[:NDVE])
    plan = {}
    for t in range(NT):
        for p in range(NPAN_T):
            plan[(t, p)] = "DVE" if (t, p) in dve else "PE"
    return plan


def build_nc(plan):
    nc = bacc.Bacc()
    det = nc.declare_dram_parameter("details", [ROWS, M], F32, isOutput=False)
    app = nc.declare_dram_parameter("approximation", [ROWS, M], F32, isOutput=False)
    wav = nc.declare_dram_parameter("wavelet", [8], F32, isOutput=False)
    res = nc.declare_dram_parameter("result", [ROWS, 2 * M], F32, isOutput=True)
    ident = nc.inline_tensor(np.eye(P, dtype=np.float32), "ident")

    with tile.TileContext(nc) as tc:
        with (
            tc.tile_pool(name="const", bufs=1) as constp,
            tc.tile_pool(name="ine", bufs=INP_BUFS) as inp,
            tc.tile_pool(name="dvine", bufs=DVE_BUFS) as dvp,
            tc.tile_pool(name="oute", bufs=OUT_BUFS) as outp,
            tc.tile_pool(name="acc", bufs=ACC_BUFS) as accp,
            tc.tile_pool(name="psum", bufs=PSUM_BUFS, space="PSUM") as psump,
            tc.tile_pool(name="psumw", bufs=1, space="PSUM") as psumw,
        ):
            # ---- coefficients: broadcast wavelet to all partitions
            wv = constp.tile([1, 8], F32)
            nc.sync.dma_start(wv[:], wav[None, :])
            wvb = constp.tile([P, 8], F32)
            nc.gpsimd.partition_broadcast(wvb[:], wv[:])
            wvn = constp.tile([P, 8], F32)
            nc.vector.tensor_scalar(wvn[:], wvb[:], -1.0, None, MUL)

            def coeff(x, pi, s):
                # x: 0 = details, 1 = approximation; pi: 0 = even, 1 = odd
                if x == 0:
                    idx = (7 - 2 * s) if pi == 0 else (6 - 2 * s)
                    return wvb[:, idx : idx + 1]
                if pi == 0:
                    idx = 2 * s
                    return wvb[:, idx : idx + 1]
                idx = 2 * s + 1
                return wvn[:, idx : idx + 1]

            # ---- PE weights: c * I for each (input, parity, tap); built on
            # ScalarE so DVE's MAC chains aren't head-blocked behind them.
            it = constp.tile([P, P], F32)
            nc.sync.dma_start(it[:], ident[:, :])
            w16 = constp.tile([P, 16 * P], CDT)

            def wslice(x, pi, s):
                j = (x * 2 + pi) * 4 + s
                return w16[:, j * P : (j + 1) * P]

            if WARMUP:
                # prime the PE p-state ramp while weights build: a few dummy
                # matmuls on the identity keep PE continuously busy so the
                # first real matmuls run at full clock
                wps = psumw.tile([P, P], F32, tag="warm")
                for _ in range(WARMUP):
                    nc.tensor.matmul(wps[:], it[:], it[:], start=True, stop=True)

            # even-parity slices on ScalarE (PE consumes them first), odd on
            # DVE (cheap there; overlaps the first panel's load latency)
            for x in range(2):
                for s in range(4):
                    nc.scalar.activation(
                        wslice(x, 0, s), it[:], COPY, scale=coeff(x, 0, s)
                    )
            for x in range(2):
                for s in range(4):
                    nc.vector.tensor_scalar(
                        wslice(x, 1, s), it[:], coeff(x, 1, s), None, MUL
                    )

            taps = [(x, s) for x in range(2) for s in range(4)]
            store_engines = [nc.sync, nc.scalar] if QSPREAD else [nc.sync]
            n_store = [0]

            if DTYPE_MODE == "f32r" and LSPREAD:
                load_engines = [nc.gpsimd, nc.sync]
            else:
                load_engines = [nc.gpsimd]
            n_load = [0]

            def load_panel(t, p, eng):
                r0 = t * P
                c0 = p * WU
                pool = dvp if eng == "DVE" else inp
                tiles = []
                for x, src in ((0, det), (1, app)):
                    ld = load_engines[n_load[0] % len(load_engines)].dma_start
                    n_load[0] += 1
                    pv = pool.tile([P, WU + 3], LDT, tag=f"pv{x}{eng}")

                    def sv(a, b):
                        v = src[r0 : r0 + P, a:b]
                        return v.bitcast(F32R) if DTYPE_MODE == "f32r" else v

                    if c0 >= 3:
                        ld(pv[:, :], sv(c0 - 3, c0 + WU))
                    else:
                        ld(pv[:, 3:], sv(c0, c0 + WU))
                        ld(pv[:, 0:3], sv(M - 3, M))
                    tiles.append(pv)
                return tiles

            def body(_i=None):
                pre_acc = {}
                preloaded = {}
                for t in range(NT):
                    r0 = t * P
                    for p in range(NPAN_T):
                        c0 = p * WU
                        eng = plan[(t, p)]
                        pv = preloaded.pop((t, p), None) or load_panel(t, p, eng)

                        def dv(x, s, off=0, n=WU):
                            v = pv[x][:, 3 - s + off : 3 - s + off + n]
                            return v.bitcast(F32) if DTYPE_MODE == "f32r" else v

                        ot = outp.tile([P, 2 * WU], F32, tag="out")
                        for pi in range(2):
                            oview = ot[:, pi : 2 * WU : 2]
                            if IO_ONLY:
                                nc.scalar.copy(oview, dv(pi, 0))
                                continue
                            if eng == "PE":
                                # chunk-major: finish+evict chunk ci while
                                # chunk ci+1's matmuls run (short PSUM dwell;
                                # ldweights reloads hide behind the matmuls)
                                for ci, cc in enumerate(range(0, WU, NCHUNK)):
                                    ps = psump.tile([P, NCHUNK], F32, tag="ps",
                                                    name=f"pz_{t}_{p}_{pi}_{ci}")
                                    for j, (x, s) in enumerate(taps):
                                        rhs = pv[x][:, 3 - s + cc : 3 - s + cc + NCHUNK]
                                        nc.tensor.matmul(
                                            ps[:], wslice(x, pi, s), rhs,
                                            start=(j == 0), stop=(j == len(taps) - 1),
                                        )
                                    nc.scalar.copy(
                                        ot[:, 2 * cc + pi : min(2 * cc + pi + 2 * NCHUNK, 2 * WU) : 2],
                                        ps[:],
                                    )
                            else:  # DVE
                                acc = pre_acc.pop((t, p, pi), None)
                                if acc is None:
                                    acc = accp.tile([P, WU], F32, tag="acc")
                                    nc.scalar.activation(
                                        acc[:], dv(0, 0), COPY, scale=coeff(0, pi, 0)
                                    )
                                for x, s in taps[1:-1]:
                                    nc.vector.scalar_tensor_tensor(
                                        acc[:], dv(x, s),
                                        coeff(x, pi, s), acc[:], MUL, ADD,
                                    )
                                x, s = taps[-1]
                                nc.vector.scalar_tensor_tensor(
                                    oview, dv(x, s),
                                    coeff(x, pi, s), acc[:], MUL, ADD,
                                )
                        sw = 2 * WU // STORE_SPLIT
                        for si, sc in enumerate(range(0, 2 * WU, sw)):
                            if eng == "DVE":
                                # never ScalarE (would head-block its
                                # eviction stream)
                                st = nc.sync
                            else:
                                st = store_engines[n_store[0] % len(store_engines)]
                                n_store[0] += 1
                            st.dma_start(
                                res[r0 : r0 + P, 2 * c0 + sc : 2 * c0 + sc + sw],
                                ot[:, sc : sc + sw],
                            )

            if REPS == 1:
                body()
            else:
                with tc.For_i(0, REPS, 1) as _rv:
                    body(_rv)
    nc.finalize()
    return nc


_CACHE = {}


def _get_nc():
    if "nc" not in _CACHE:
        _CACHE["nc"] = build_nc(_unit_plan())
    return _CACHE["nc"]


def kernel(details, approximation, wavelet):
    details = np.ascontiguousarray(np.asarray(details, dtype=np.float32))
    approximation = np.ascontiguousarray(np.asarray(approximation, dtype=np.float32))
    wavelet = np.ascontiguousarray(np.asarray(wavelet, dtype=np.float32))
    assert details.shape == (N_CORES * ROWS, M) and approximation.shape == details.shape
    assert wavelet.shape == (8,)

    in_maps = [
        {
            "details": details[c * ROWS : (c + 1) * ROWS],
            "approximation": approximation[c * ROWS : (c + 1) * ROWS],
            "wavelet": wavelet,
        }
        for c in range(N_CORES)
    ]
    trace = bool(int(os.environ.get("DWT_TRACE", "0")))
    r = run_bass_kernel_spmd(_get_nc(), in_maps, list(range(N_CORES)), trace=trace)
    _CACHE["last_results"] = r
    return np.concatenate([r.results[c]["result"] for c in range(N_CORES)], axis=0)


# revision 54
# speedup vs baseline: 1.4103x; 1.0237x over previous
"""Inverse DWT (BackwardTransformLayer) Trainium2 Bass kernel, v2.

Math (polyphase form of the zero-interleaved circular FFT convolution):
  out[r, 2p+pi] = sum_{s=0..3} cD[pi,s]*D[r,(p-s)%M] + cA[pi,s]*A[r,(p-s)%M]
  cD[0,s] = w[7-2s]   cD[1,s] = w[6-2s]   cA[0,s] = w[2s]   cA[1,s] = -w[2s+1]

Sharding: data-parallel over rows; 512 rows per core on 8 NeuronCores
(run_bass_kernel_spmd). Full fp32 inputs in, full fp32 output out.

v2 structure (vs the v1 full-row-tile kernel): all input traffic is
per-panel [128, WU+3] tiles DMA'd straight from DRAM with the 3-column
circular prefix included in the transfer (panel 0 adds a tiny wrap DMA).
This removes v1's on-chip prefix copies and the private-DVE-copy
duplication (-5 MiB DMA/core), starts compute earlier, and gives every
work unit a private SBUF tile (no PE/DVE SBUF-port contention, which is
what made v1's sim model underpredict HW by 15%). 16 panels of WU=1024
cols; 5 on DVE (placed early; DVE is slower per panel), 11 on PE; each
panel's two parity units write one [128, 2*WU] output tile stored in
halves as soon as both finish.
  - PE panels: 8 identity-scaled f32r matmuls accumulate in a PSUM bank
    per 512-col chunk (chunk-major so ScalarE evicts chunk 0 stride-2
    into the output tile while chunk 1 computes). The f32r DRAM-side
    bitcast happens in the DMA (BIR requires f32r matmul inputs to be
    produced as f32r), so loads work on any DGE queue.
  - DVE panels: ScalarE first tap, then scalar_tensor_tensor MAC chain,
    last MAC written stride-2 into the output tile.
  - Weights: even-parity c*I slices built on ScalarE, odd on DVE; a few
    dummy identity matmuls prime the PE p-state ramp.
  - Queues: loads alternate qPool (SWDGE)/qSP, stores alternate qSP/qAct
    (DVE-panel stores qSP only: on qAct they would head-block ScalarE's
    eviction stream; on qPool they regress HW by 25us+).

Measured (test.py REPS-slope, all 8 cores): 95.8us best / ~120-123us
typical across semantically-identical rebuilds (schedule-lottery
sensitivity; CoreSim predicts 90-95us for all of them), vs 124-128us
for v1 and ~202-210us for the fp32 reference port. rel err 1.75e-4
(f32r mantissa) vs the 2e-2 gate. Negative results: bf16 loads+compute
117.8us (despite halving modeled DMA; DVE 16-bit 2x modes do not engage
on HW), WU=512 (per-op overhead), inline per-chunk PE stores, moving
weight builds/first-taps into the body, SWDGE stores.
"""

import os
import sys

import numpy as np

for _p in ("/opt/trn_rl_repo", "/root/.axon_site/_ro/trn_rl_repo"):
    if os.path.isdir(_p) and _p not in sys.path:
        sys.path.append(_p)

import concourse.bass as bass  # noqa: E402
import concourse.tile as tile  # noqa: E402
from concourse import bacc, mybir  # noqa: E402
from concourse.bass_utils import run_bass_kernel_spmd  # noqa: E402

F32 = mybir.dt.float32
F32R = mybir.dt.float32r
BF16 = mybir.dt.bfloat16
COPY = mybir.ActivationFunctionType.Copy
MUL = mybir.AluOpType.mult
ADD = mybir.AluOpType.add

N_CORES = 8
P = 128          # partitions
M = 4096         # input row length
ROWS = 512       # rows per core
NT = ROWS // P   # row tiles per core
WU = int(os.environ.get("DWT_WU", "1024"))   # panel width (input cols)
NPAN_T = M // WU                             # panels per row tile
NCHUNK = 512                                 # psum chunk (one fp32 bank)
NDVE = int(os.environ.get("DWT_NDVE", "5"))  # panels on DVE (of NT*NPAN_T)
INP_BUFS = int(os.environ.get("DWT_INP_BUFS", "8"))   # PE input ring (per input)
DVE_BUFS = int(os.environ.get("DWT_DVE_BUFS", "5"))   # DVE input ring (per input)
WARMUP = int(os.environ.get("DWT_WARMUP", "6"))  # dummy matmuls to ramp PE pstate
OUT_BUFS = int(os.environ.get("DWT_OUT_BUFS", "5"))
ACC_BUFS = int(os.environ.get("DWT_ACC_BUFS", "4"))
PSUM_BUFS = int(os.environ.get("DWT_PSUM_BUFS", "7"))  # +1 bank for PE warmup
STORE_SPLIT = int(os.environ.get("DWT_STORE_SPLIT", "1"))  # DMAs per out tile
REPS = int(os.environ.get("DWT_REPS", "1"))  # benchmark-only in-kernel loop
IO_ONLY = bool(int(os.environ.get("DWT_IO_ONLY", "0")))
DTYPE_MODE = os.environ.get("DWT_DTYPE", "f32r")  # f32r | bf16
QSPREAD = bool(int(os.environ.get("DWT_QSPREAD", "1")))  # stores qSP+qAct
CDT = {"f32r": F32R, "bf16": BF16}[DTYPE_MODE]   # matmul/weight dtype
# Input tiles carry the matmul dtype; in f32r mode the DRAM source AP is
# bitcast to f32r so the transfer is dtype-matched on any DGE queue (the BIR
# verifier requires f32r matmul inputs to be produced as f32r, so the tile
# itself must be f32r). bf16 needs the SWDGE downconvert path.
LDT = CDT
LSPREAD = bool(int(os.environ.get("DWT_LSPREAD", "1")))  # loads qPool+qSP


def _unit_plan():
    """Panel -> engine map. DVE panels are placed early in the pipeline
    (DVE is the slower engine per panel; its last unit must not define the
    tail), but panel (0,0) stays on PE so PE's first work unit is the very
    first panel loaded."""
    force = os.environ.get("DWT_FORCE_ENG")
    if force:
        return {(t, p): force for t in range(NT) for p in range(NPAN_T)}
    order = [(0, 1)] + [(t, 0) for t in range(1, NT)]
    order += [(t, p) for p in range(1, NPAN_T) for t in range(NT) if (t, p) != (0, 1)]
    order += [(0, 0)]
    dve = set(order[:NDVE])
    plan = {}
    for t in range(NT):
        for p in range(NPAN_T):
            plan[(t, p)] = "DVE" if (t, p) in dve else "PE"
    return plan


def build_nc(plan):
    nc = bacc.Bacc()
    det = nc.declare_dram_parameter("details", [ROWS, M], F32, isOutput=False)
    app = nc.declare_dram_parameter("approximation", [ROWS, M], F32, isOutput=False)
    wav = nc.declare_dram_parameter("wavelet", [8], F32, isOutput=False)
    res = nc.declare_dram_parameter("result", [ROWS, 2 * M], F32, isOutput=True)
    ident = nc.inline_tensor(np.eye(P, dtype=np.float32), "ident")

    with tile.TileContext(nc) as tc:
        with (
            tc.tile_pool(name="const", bufs=1) as constp,
            tc.tile_pool(name="ine", bufs=INP_BUFS) as inp,
            tc.tile_pool(name="dvine", bufs=DVE_BUFS) as dvp,
            tc.tile_pool(name="oute", bufs=OUT_BUFS) as outp,
            tc.tile_pool(name="acc", bufs=ACC_BUFS) as accp,
            tc.tile_pool(name="psum", bufs=PSUM_BUFS, space="PSUM") as psump,
            tc.tile_pool(name="psumw", bufs=1, space="PSUM") as psumw,
        ):
            # ---- coefficients: broadcast wavelet to all partitions
            wv = constp.tile([1, 8], F32)
            nc.sync.dma_start(wv[:], wav[None, :])
            wvb = constp.tile([P, 8], F32)
            nc.gpsimd.partition_broadcast(wvb[:], wv[:])
            wvn = constp.tile([P, 8], F32)
            nc.vector.tensor_scalar(wvn[:], wvb[:], -1.0, None, MUL)

            def coeff(x, pi, s):
                # x: 0 = details, 1 = approximation; pi: 0 = even, 1 = odd
                if x == 0:
                    idx = (7 - 2 * s) if pi == 0 else (6 - 2 * s)
                    return wvb[:, idx : idx + 1]
                if pi == 0:
                    idx = 2 * s
                    return wvb[:, idx : idx + 1]
                idx = 2 * s + 1
                return wvn[:, idx : idx + 1]

            # ---- PE weights: c * I for each (input, parity, tap); built on
            # ScalarE so DVE's MAC chains aren't head-blocked behind them.
            it = constp.tile([P, P], F32)
            nc.sync.dma_start(it[:], ident[:, :])
            w16 = constp.tile([P, 16 * P], CDT)

            def wslice(x, pi, s):
                j = (x * 2 + pi) * 4 + s
                return w16[:, j * P : (j + 1) * P]

            if WARMUP:
                # prime the PE p-state ramp while weights build: a few dummy
                # matmuls on the identity keep PE continuously busy so the
                # first real matmuls run at full clock
                wps = psumw.tile([P, P], F32, tag="warm")
                for _ in range(WARMUP):
                    nc.tensor.matmul(wps[:], it[:], it[:], start=True, stop=True)

            # even-parity slices on ScalarE (PE consumes them first), odd on
            # DVE (cheap there; overlaps the first panel's load latency)
            for x in range(2):
                for s in range(4):
                    nc.scalar.activation(
                        wslice(x, 0, s), it[:], COPY, scale=coeff(x, 0, s)
                    )
            for x in range(2):
                for s in range(4):
                    nc.vector.tensor_scalar(
                        wslice(x, 1, s), it[:], coeff(x, 1, s), None, MUL
                    )

            taps = [(x, s) for x in range(2) for s in range(4)]
            store_engines = [nc.sync, nc.scalar] if QSPREAD else [nc.sync]
            n_store = [0]

            if DTYPE_MODE == "f32r" and LSPREAD:
                load_engines = [nc.gpsimd, nc.sync]
            else:
                load_engines = [nc.gpsimd]
            n_load = [0]

            def load_panel(t, p, eng):
                r0 = t * P
                c0 = p * WU
                pool = dvp if eng == "DVE" else inp
                tiles = []
                for x, src in ((0, det), (1, app)):
                    ld = load_engines[n_load[0] % len(load_engines)].dma_start
                    n_load[0] += 1
                    pv = pool.tile([P, WU + 3], LDT, tag=f"pv{x}{eng}")

                    def sv(a, b):
                        v = src[r0 : r0 + P, a:b]
                        return v.bitcast(F32R) if DTYPE_MODE == "f32r" else v

                    if c0 >= 3:
                        ld(pv[:, :], sv(c0 - 3, c0 + WU))
                    else:
                        ld(pv[:, 3:], sv(c0, c0 + WU))
                        ld(pv[:, 0:3], sv(M - 3, M))
                    tiles.append(pv)
                return tiles

            def body(_i=None):
                pre_acc = {}
                preloaded = {}
                for t in range(NT):
                    r0 = t * P
                    for p in range(NPAN_T):
                        c0 = p * WU
                        eng = plan[(t, p)]
                        pv = preloaded.pop((t, p), None) or load_panel(t, p, eng)

                        def dv(x, s, off=0, n=WU):
                            v = pv[x][:, 3 - s + off : 3 - s + off + n]
                            return v.bitcast(F32) if DTYPE_MODE == "f32r" else v

                        ot = outp.tile([P, 2 * WU], F32, tag="out")
                        for pi in range(2):
                            oview = ot[:, pi : 2 * WU : 2]
                            if IO_ONLY:
                                nc.scalar.copy(oview, dv(pi, 0))
                                continue
                            if eng == "PE":
                                # chunk-major: finish+evict chunk ci while
                                # chunk ci+1's matmuls run (short PSUM dwell;
                                # ldweights reloads hide behind the matmuls)
                                for ci, cc in enumerate(range(0, WU, NCHUNK)):
                                    ps = psump.tile([P, NCHUNK], F32, tag="ps",
                                                    name=f"pz_{t}_{p}_{pi}_{ci}")
                                    for j, (x, s) in enumerate(taps):
                                        rhs = pv[x][:, 3 - s + cc : 3 - s + cc + NCHUNK]
                                        nc.tensor.matmul(
                                            ps[:], wslice(x, pi, s), rhs,
                                            start=(j == 0), stop=(j == len(taps) - 1),
                                        )
                                    nc.scalar.copy(
                                        ot[:, 2 * cc + pi : min(2 * cc + pi + 2 * NCHUNK, 2 * WU) : 2],
                                        ps[:],
                                    )
                            else:  # DVE
                                acc = pre_acc.pop((t, p, pi), None)
                                if acc is None:
                                    acc = accp.tile([P, WU], F32, tag="acc")
                                    nc.scalar.activation(
                                        acc[:], dv(0, 0), COPY, scale=coeff(0, pi, 0)
                                    )
                                for x, s in taps[1:-1]:
                                    nc.vector.scalar_tensor_tensor(
                                        acc[:], dv(x, s),
                                        coeff(x, pi, s), acc[:], MUL, ADD,
                                    )
                                x, s = taps[-1]
                                nc.vector.scalar_tensor_tensor(
                                    oview, dv(x, s),
                                    coeff(x, pi, s), acc[:], MUL, ADD,
                                )
                        sw = 2 * WU // STORE_SPLIT
                        for si, sc in enumerate(range(0, 2 * WU, sw)):
                            if eng == "DVE":
                                # never ScalarE (would head-block its
                                # eviction stream)
                                st = nc.sync
                            else:
                                st = store_engines[n_store[0] % len(store_engines)]
                                n_store[0] += 1
                            st.dma_start(
                                res[r0 : r0 + P, 2 * c0 + sc : 2 * c0 + sc + sw],
                                ot[:, sc : sc + sw],
                            )

            if REPS == 1:
                body()
            else:
                with tc.For_i(0, REPS, 1) as _rv:
                    body(_rv)
    nc.finalize()
    return nc


_CACHE = {}


def _get_nc():
    if "nc" not in _CACHE:
        _CACHE["nc"] = build_nc(_unit_plan())
    return _CACHE["nc"]


def kernel(details, approximation, wavelet):
    details = np.ascontiguousarray(np.asarray(details, dtype=np.float32))
    approximation = np.ascontiguousarray(np.asarray(approximation, dtype=np.float32))
    wavelet = np.ascontiguousarray(np.asarray(wavelet, dtype=np.float32))
    assert details.shape == (N_CORES * ROWS, M) and approximation.shape == details.shape
    assert wavelet.shape == (8,)

    in_maps = [
        {
            "details": details[c * ROWS : (c + 1) * ROWS],
            "approximation": approximation[c * ROWS : (c + 1) * ROWS],
            "wavelet": wavelet,
        }
        for c in range(N_CORES)
    ]
    trace = bool(int(os.environ.get("DWT_TRACE", "0")))
    r = run_bass_kernel_spmd(_get_nc(), in_maps, list(range(N_CORES)), trace=trace)
    _CACHE["last_results"] = r
    return np.concatenate([r.results[c]["result"] for c in range(N_CORES)], axis=0)
